# revision 12
# baseline (speedup 1.0000x reference)
"""Trainium2 Bass kernel for BinarySplitDecoder (binary-tree leaf probabilities).

Contract: kernel(x) takes the FULL input x [65536, 1023] fp32 and returns the
FULL output [65536, 1024] fp32 (leaf probabilities of a depth-10 binary split
tree, level-major node ordering).

Sharding: pure data parallel — batch dim split evenly across 8 NeuronCores.

Per-core kernel layout:
  - rows_per_core = 8192, processed in chunks of G*128 rows.
  - Within a chunk, partition p / free-group g holds batch row  g*128 + p.
  - DVE computes oma = 1 - x for the whole chunk in one tensor_scalar op
    (fp32 single-src SBUF runs in 2x mode), then the tree level by level:
    left = cur * a ; right = cur * oma, written interleaved (stride 2) into
    the next level's tile. fp32 tensor_tensor runs in 1x DVE mode regardless
    of stride, so the interleaved store is free. This matches the reference's
    fp32 operation sequence exactly (no cancellation error on small leaves).
    Keeping all compute on one engine also keeps every DMA's wait set at
    <=2 semaphores (the HW DMA instruction has only 2 sync-wait slots).
"""

import numpy as np

import concourse.bacc as bacc
import concourse.bass as bass
import concourse.mybir as mybir
from concourse.tile import TileContext
from concourse.bass_utils import run_bass_kernel_spmd

TREE_DEPTH = 10
N_NODES = (1 << TREE_DEPTH) - 1  # 1023
N_LEAVES = 1 << TREE_DEPTH  # 1024
N_CORES = 8
P = 128  # SBUF partitions


def build_nc(rows_per_core: int, G: int = 8) -> bass.Bass:
    """Build the per-core Bass program.

    rows_per_core must be divisible by G*128. The kernel reads DRAM input
    "x" [rows_per_core, 1023] and writes "y" [rows_per_core, 1024].
    """
    chunk_rows = G * P
    assert rows_per_core % chunk_rows == 0
    n_chunks = rows_per_core // chunk_rows
    f32 = mybir.dt.float32

    # Bacc (not raw Bass): Bacc.compile() runs generate_event_semaphores,
    # which splits multi-wait sync onto EventSemaphore instructions (TRN2
    # instructions have a single sync-wait slot).
    nc = bacc.Bacc("TRN2", target_bir_lowering=False, debug=False)
    x = nc.declare_dram_parameter("x", [rows_per_core, N_NODES], f32, isOutput=False)
    y = nc.declare_dram_parameter("y", [rows_per_core, N_LEAVES], f32, isOutput=True)

    # [chunk, partition, group, nodes/leaves] views of DRAM. Row mapping is
    # row = c*(P*G) + p*G + g so each chunk's DMA is a contiguous
    # [128, G*width] 2D block (one run of G*width elements per partition).
    xv = x.rearrange("(c p g) n -> c p (g n)", g=G, p=P)
    yv = y.rearrange("(c p g) m -> c p (g m)", g=G, p=P)

    with TileContext(nc) as tc:
        with (
            tc.tile_pool(name="io", bufs=2) as iop,
            tc.tile_pool(name="oma", bufs=1) as omap,
            tc.tile_pool(name="cur", bufs=1) as curp,
        ):
            for c in range(n_chunks):
                xt = iop.tile([P, G, N_NODES], f32, tag="x")
                nc.sync.dma_start(out=xt[:], in_=xv[c])

                # oma = 1 - x for the whole chunk in one DVE tensor_scalar
                # (2x mode for single-src fp32 SBUF).
                oma_t = omap.tile([P, G, N_NODES], f32, tag="oma")
                nc.vector.tensor_scalar(
                    out=oma_t[:],
                    in0=xt[:],
                    scalar1=-1.0,
                    scalar2=1.0,
                    op0=mybir.AluOpType.mult,
                    op1=mybir.AluOpType.add,
                )

                out_t = iop.tile([P, G, N_LEAVES], f32, tag="y")
                cur = None
                for d in range(TREE_DEPTH):
                    L = 1 << d
                    if d == TREE_DEPTH - 1:
                        nxt = out_t
                    else:
                        # ping-pong intermediate levels between two shared
                        # slots (sized by the largest level using each tag)
                        nxt = curp.tile(
                            [P, G, 2 * L], f32, tag=f"cur{d % 2}"
                        )
                    a = xt[:, :, L - 1 : 2 * L - 1]  # [P, G, L] this level's alphas
                    oma = oma_t[:, :, L - 1 : 2 * L - 1]
                    left = nxt[:, :, 0::2]
                    right = nxt[:, :, 1::2]
                    if d == 0:
                        # cur == 1:  left = a, right = 1 - a
                        nc.vector.tensor_copy(out=left, in_=a)
                        nc.vector.tensor_copy(out=right, in_=oma)
                    else:
                        nc.vector.tensor_mul(out=left, in0=cur, in1=a)
                        nc.vector.tensor_mul(out=right, in0=cur, in1=oma)
                    cur = nxt

                nc.sync.dma_start(out=yv[c], in_=out_t[:])

    nc.compile()
    return nc


def _run(x: np.ndarray, **spmd_kwargs):
    """Shard x, run the Bass kernel on all 8 cores, return (y, BassKernelResults)."""
    x = np.ascontiguousarray(np.asarray(x, dtype=np.float32))
    B = x.shape[0]
    assert B % N_CORES == 0 and x.shape[1] == N_NODES
    rows_per_core = B // N_CORES

    nc = build_nc(rows_per_core)
    core_ids = list(range(N_CORES))
    in_maps = [
        {"x": x[i * rows_per_core : (i + 1) * rows_per_core]} for i in core_ids
    ]
    res = run_bass_kernel_spmd(nc, in_maps, core_ids, **spmd_kwargs)
    out = np.concatenate([r["y"] for r in res.results], axis=0)
    return out, res


def kernel(x: np.ndarray) -> np.ndarray:
    return _run(x)[0]
